# revision 6
# baseline (speedup 1.0000x reference)
"""Adaptive tied-input softmax on 8 TRN2 NeuronCores.

Sharding: vocab-parallel logits (each core owns 1/8 of each bucket's rows),
token-parallel embeddings (128 tokens/core). No collectives — the host
assembles the full outputs from per-core shards.

Per-core device graph (identical across cores, SPMD):
  - all weights DMA'd into SBUF once (~13 MB bf16, resident)
  - emb   = gathered+masked rows (host gather) @ stacked proj weights
  - t_l^T = projs[l]^T @ hidden^T            (tail projections, transposed)
  - head logits = hidden @ [e0_shard | cls_w]^T  (cls_w folded in -> l_tail)
  - tail logits = t_l @ e_l_shard^T + l_tail[l-1] (bias fused into eviction)
  - logits evicted from PSUM with f32->bf16 cast, DMA'd out per 128-token row
"""

import numpy as np
import ml_dtypes

import concourse.bacc as bacc
import concourse.mybir as mybir
import concourse.tile as tile
from concourse.bass_utils import run_bass_kernel_spmd

BF16 = ml_dtypes.bfloat16
DT_BF16 = mybir.dt.bfloat16
DT_F32 = mybir.dt.float32

# ---- problem constants (hardcoded; kernel.py must be self-contained) ----
DIM = 1024
VOCABS = [20000, 20000, 30000, 30000]
VDS = [1024, 256, 64, 16]
VOFF = [0, 20000, 40000, 70000]
B, S = 2, 512
NTOK = B * S  # 1024
NCORES = 8
TPC = NTOK // NCORES  # 128 tokens/core for embeddings
VS = [v // NCORES for v in VOCABS]  # per-core vocab shard: 2500,2500,3750,3750
HEADC = VS[0] + 4  # head matmul cols: 2500 vocab + 3 cls + 1 pad
LCOLS = sum(VS)  # 12500 logit cols per core
KG = 11  # gather K-chunks: 1360 -> pad 1408 = 11*128
KPAD = KG * 128
GCOL0 = [0, 1024, 1280, 1344]  # col starts of each bucket in gathered G
PCOL0 = [None, 0, 256, 320]  # col starts of projs[l] in pall (width 336)
PALLC = 336

TRACE = False
LAST_EXEC_TIME_NS = None
LAST_TRACE_PATH = None

_CACHE = {}


def _install_trace_shim():
    """Register the NTFF profile hook that bass_utils expects under axon.

    The agent image's antenv package lacks ``axon_hooks``; provide an
    in-process stand-in and register the ctypes-driven hook from
    trn_agent_boot. Also neutralize the artifact upload (no bucket here).
    """
    import sys
    import types

    if "antenv.axon_hooks" not in sys.modules:
        mod = types.ModuleType("antenv.axon_hooks")
        _hook = [None]
        mod.set_axon_ntff_profile_hook = lambda h: _hook.__setitem__(0, h)
        mod.get_axon_ntff_profile_hook = lambda: _hook[0]
        sys.modules["antenv.axon_hooks"] = mod
        try:
            from trn_agent_boot.trn_boot import _ntff_profile_via_ctypes

            mod.set_axon_ntff_profile_hook(
                _ntff_profile_via_ctypes("/opt/axon/libaxon_pjrt.so")
            )
        except Exception:
            pass
    import concourse.bass_utils as _bu

    _bu.upload_artifacts = lambda tmpdir: tmpdir


def _chunks(total, size):
    return [(o, min(size, total - o)) for o in range(0, total, size)]


def _build(cls_b):
    nc = bacc.Bacc(None, target_bir_lowering=False, debug=False)

    d_hT = nc.declare_dram_parameter("hT", [128, 8, NTOK], DT_BF16, False)
    d_e0T = nc.declare_dram_parameter("e0T", [128, 8, HEADC], DT_BF16, False)
    d_e1T = nc.declare_dram_parameter("e1T", [128, 2, VS[1]], DT_BF16, False)
    d_e2T = nc.declare_dram_parameter("e2T", [64, VS[2]], DT_BF16, False)
    d_e3T = nc.declare_dram_parameter("e3T", [16, VS[3]], DT_BF16, False)
    d_pall = nc.declare_dram_parameter("pall", [128, 8, PALLC], DT_BF16, False)
    d_gT = nc.declare_dram_parameter("gT", [128, KG, TPC], DT_BF16, False)
    d_wemb = nc.declare_dram_parameter("wemb", [128, KG, DIM], DT_BF16, False)
    d_logits = nc.declare_dram_parameter("logits", [NTOK, LCOLS], DT_BF16, True)
    d_emb = nc.declare_dram_parameter("emb", [TPC, DIM], DT_F32, True)

    ADD = mybir.AluOpType.add

    with tile.TileContext(nc) as tc:
        with (
            tc.tile_pool(name="w", bufs=1) as wpool,
            tc.tile_pool(name="k", bufs=1) as kpool,
            tc.tile_pool(name="stage", bufs=2) as spool,
            tc.tile_pool(name="ps", bufs=8, space="PSUM") as pspool,
        ):
            hT = wpool.tile([128, 8, NTOK], DT_BF16)
            e0T = wpool.tile([128, 8, HEADC], DT_BF16)
            e1T = wpool.tile([128, 2, VS[1]], DT_BF16)
            e2T = wpool.tile([64, VS[2]], DT_BF16)
            e3T = wpool.tile([16, VS[3]], DT_BF16)
            pall = wpool.tile([128, 8, PALLC], DT_BF16)
            gT = wpool.tile([128, KG, TPC], DT_BF16)
            wemb = wpool.tile([128, KG, DIM], DT_BF16)

            t1 = kpool.tile([128, 2, NTOK], DT_BF16)
            t2 = kpool.tile([64, NTOK], DT_BF16)
            t3 = kpool.tile([16, NTOK], DT_BF16)
            ltail = kpool.tile([128, 8, 4], DT_F32)
            clsb = kpool.tile([128, 4], DT_F32)
            embsb = kpool.tile([TPC, DIM], DT_F32)

            # input DMAs on the SP HWDGE queue, ordered so early-phase deps
            # arrive first (outputs go on the ACT/Pool queues to not contend)
            nc.sync.dma_start(gT[:], d_gT[:])
            nc.sync.dma_start(wemb[:], d_wemb[:])
            nc.sync.dma_start(hT[:], d_hT[:])
            nc.sync.dma_start(pall[:], d_pall[:])
            nc.sync.dma_start(e0T[:], d_e0T[:])
            nc.sync.dma_start(e1T[:], d_e1T[:])
            nc.sync.dma_start(e2T[:], d_e2T[:])
            nc.sync.dma_start(e3T[:], d_e3T[:])

            for j in range(3):
                nc.vector.memset(clsb[:, j : j + 1], float(cls_b[j]))
            nc.vector.memset(clsb[:, 3:4], 0.0)

            # PE warmup: dense dummy matmuls so the HAM clock-gate reaches
            # 8/8 while the input DMAs stream in (PE would otherwise idle
            # cold for ~12us and pay the 1.2 GHz throttle on real work).
            wdum = kpool.tile([128, 512], DT_BF16)
            wsink = kpool.tile([128, 512], DT_F32)
            nc.vector.memset(wdum[:], 0.0)
            wps = pspool.tile([128, 512], DT_F32, tag="ps")
            for i in range(44):
                nc.tensor.matmul(wps[:], wdum[:, :128], wdum[:], start=True, stop=True)
            nc.vector.tensor_copy(wsink[:], wps[:])

            # ---- embeddings: emb = G @ W  (K = 1408, M = 128 tok, N = 1024)
            for n in range(2):
                ps = pspool.tile([128, 512], DT_F32, tag="ps")
                for k in range(KG):
                    nc.tensor.matmul(
                        ps[:],
                        gT[:, k, :],
                        wemb[:, k, n * 512 : (n + 1) * 512],
                        start=(k == 0),
                        stop=(k == KG - 1),
                    )
                nc.vector.tensor_copy(embsb[:, n * 512 : (n + 1) * 512], ps[:])
            nc.gpsimd.dma_start(d_emb[:], embsb[:])

            # ---- t_l^T = projs[l]^T @ hidden^T  -> [vd_l, NTOK] bf16
            for mj in range(2):  # l=1, vd=256 -> two 128-row chunks
                for n in range(2):
                    ps = pspool.tile([128, 512], DT_F32, tag="ps")
                    for k in range(8):
                        nc.tensor.matmul(
                            ps[:],
                            pall[:, k, mj * 128 : (mj + 1) * 128],
                            hT[:, k, n * 512 : (n + 1) * 512],
                            start=(k == 0),
                            stop=(k == 7),
                        )
                    nc.vector.tensor_copy(t1[:, mj, n * 512 : (n + 1) * 512], ps[:])
            for tdst, lo, hi in ((t2, 256, 320), (t3, 320, 336)):
                mrows = hi - lo
                for n in range(2):
                    ps = pspool.tile([128, 512], DT_F32, tag="ps")
                    for k in range(8):
                        nc.tensor.matmul(
                            ps[:mrows, :],
                            pall[:, k, lo:hi],
                            hT[:, k, n * 512 : (n + 1) * 512],
                            start=(k == 0),
                            stop=(k == 7),
                        )
                    nc.vector.tensor_copy(tdst[:, n * 512 : (n + 1) * 512], ps[:mrows, :])

            # ---- main logits loop, one 128-token row-block at a time
            hchunks = [(2000, 504)] + _chunks(2000, 500)  # cls chunk first
            for m in range(8):
                ms = slice(m * 128, (m + 1) * 128)
                stg_h = spool.tile([128, VS[0]], DT_BF16, tag="stage_h")
                stg_t = spool.tile([128, LCOLS - VS[0]], DT_BF16, tag="stage_t")

                # head (+ fold-in of cls_w -> l_tail)
                for off, n_ in hchunks:
                    ps = pspool.tile([128, 512], DT_F32, tag="ps")
                    for k in range(8):
                        nc.tensor.matmul(
                            ps[:, :n_],
                            hT[:, k, ms],
                            e0T[:, k, off : off + n_],
                            start=(k == 0),
                            stop=(k == 7),
                        )
                    if n_ == 504:
                        nc.vector.tensor_copy(stg_h[:, 2000:2500], ps[:, :500])
                        nc.vector.tensor_tensor(ltail[:, m, :], ps[:, 500:504], clsb[:], op=ADD)
                    else:
                        nc.vector.tensor_copy(stg_h[:, off : off + n_], ps[:, :n_])
                nc.scalar.dma_start(d_logits[ms, 0 : VS[0]], stg_h[:])

                # tail l=1 (K=256): bias add on ScalarE during eviction
                for off, n_ in _chunks(VS[1], 500):
                    ps = pspool.tile([128, 512], DT_F32, tag="ps")
                    for k in range(2):
                        nc.tensor.matmul(
                            ps[:, :n_],
                            t1[:, k, ms],
                            e1T[:, k, off : off + n_],
                            start=(k == 0),
                            stop=(k == 1),
                        )
                    nc.scalar.add(stg_t[:, off : off + n_], ps[:, :n_], ltail[:, m, 0:1])

                # tails l=2,3 (K=64/16): single matmul per chunk
                for tsrc, esrc, base, bcol in ((t2, e2T, 2500, 1), (t3, e3T, 6250, 2)):
                    for i, (off, n_) in enumerate(_chunks(VS[2], 512)):
                        ps = pspool.tile([128, 512], DT_F32, tag="ps")
                        nc.tensor.matmul(
                            ps[:, :n_], tsrc[:, ms], esrc[:, off : off + n_], start=True, stop=True
                        )
                        dst = stg_t[:, base + off : base + off + n_]
                        if i % 2 == 0:
                            nc.vector.tensor_scalar_add(dst, ps[:, :n_], ltail[:, m, bcol : bcol + 1])
                        else:
                            nc.scalar.add(dst, ps[:, :n_], ltail[:, m, bcol : bcol + 1])
                nc.gpsimd.dma_start(d_logits[ms, VS[0] : LCOLS], stg_t[:])

    nc.finalize()
    return nc


def _arrange(a2d, nk):
    """[nk*128, C] row-major -> [128, nk, C] (partition-major for SBUF)."""
    c = a2d.shape[1]
    return np.ascontiguousarray(a2d.reshape(nk, 128, c).transpose(1, 0, 2))


def kernel(input, hidden, embeds, projs, cls_w, cls_b):
    idx = np.asarray(input).astype(np.int64).reshape(-1)
    hidden = np.asarray(hidden, dtype=np.float32).reshape(NTOK, DIM)
    embeds = [np.asarray(e, dtype=np.float32) for e in embeds]
    projs = [np.asarray(p, dtype=np.float32) for p in projs]
    cls_w = np.asarray(cls_w, dtype=np.float32)
    cls_b = np.asarray(cls_b, dtype=np.float32)

    emb_bf = [e.astype(BF16) for e in embeds]
    proj_bf = [p.astype(BF16) for p in projs]
    cls_w_bf = cls_w.astype(BF16)

    # shared across cores
    hT_a = _arrange(np.ascontiguousarray(hidden.T).astype(BF16), 8)
    pall = np.zeros((DIM, PALLC), BF16)
    pall[:, 0:256] = proj_bf[1]
    pall[:, 256:320] = proj_bf[2]
    pall[:, 320:336] = proj_bf[3]
    pall_a = _arrange(pall, 8)
    W = np.zeros((KPAD, DIM), BF16)
    for l in range(4):
        W[GCOL0[l] : GCOL0[l] + VDS[l]] = proj_bf[l].T
    wemb_a = _arrange(W, KG)

    # host-side bucket-routed gather of raw embedding rows (masked, padded)
    bucket = np.searchsorted(np.asarray(VOFF[1:] + [100000]), idx, side="right")
    G = np.zeros((NTOK, KPAD), BF16)
    for l in range(4):
        sel = bucket == l
        rows = np.clip(idx - VOFF[l], 0, VOCABS[l] - 1)
        g = emb_bf[l][rows]
        g[~sel] = 0
        G[:, GCOL0[l] : GCOL0[l] + VDS[l]] = g

    key = cls_b.tobytes()
    if key not in _CACHE:
        _CACHE[key] = _build(cls_b)
    nc = _CACHE[key]

    in_maps = []
    for c in range(NCORES):
        E0ext = np.concatenate(
            [emb_bf[0][VS[0] * c : VS[0] * (c + 1)], cls_w_bf, np.zeros((1, DIM), BF16)], axis=0
        )  # [2504, DIM]
        in_maps.append(
            {
                "hT": hT_a,
                "e0T": _arrange(np.ascontiguousarray(E0ext.T), 8),
                "e1T": _arrange(
                    np.ascontiguousarray(emb_bf[1][VS[1] * c : VS[1] * (c + 1)].T), 2
                ),
                "e2T": np.ascontiguousarray(emb_bf[2][VS[2] * c : VS[2] * (c + 1)].T),
                "e3T": np.ascontiguousarray(emb_bf[3][VS[3] * c : VS[3] * (c + 1)].T),
                "pall": pall_a,
                "gT": _arrange(np.ascontiguousarray(G[TPC * c : TPC * (c + 1)].T), KG),
                "wemb": wemb_a,
            }
        )

    global LAST_EXEC_TIME_NS, LAST_TRACE_PATH
    if TRACE:
        _install_trace_shim()
    res = run_bass_kernel_spmd(nc, in_maps, core_ids=list(range(NCORES)), trace=TRACE)
    LAST_EXEC_TIME_NS = res.exec_time_ns
    if res.instructions_and_trace is not None:
        LAST_TRACE_PATH = res.instructions_and_trace[1]

    logits = np.empty((NTOK, sum(VOCABS)), np.float32)
    emb = np.empty((NTOK, DIM), np.float32)
    for c in range(NCORES):
        lg = np.asarray(res.results[c]["logits"]).astype(np.float32)
        logits[:, VS[0] * c : VS[0] * (c + 1)] = lg[:, 0 : VS[0]]
        logits[:, VOFF[1] + VS[1] * c : VOFF[1] + VS[1] * (c + 1)] = lg[:, VS[0] : VS[0] + VS[1]]
        logits[:, VOFF[2] + VS[2] * c : VOFF[2] + VS[2] * (c + 1)] = lg[:, 5000:8750]
        logits[:, VOFF[3] + VS[3] * c : VOFF[3] + VS[3] * (c + 1)] = lg[:, 8750:12500]
        emb[TPC * c : TPC * (c + 1)] = np.asarray(res.results[c]["emb"])
    return emb.reshape(B, S, DIM), logits.reshape(B, S, sum(VOCABS))


# revision 9
# speedup vs baseline: 1.1479x; 1.1479x over previous
"""Adaptive tied-input softmax on 8 TRN2 NeuronCores.

Sharding: vocab-parallel logits (each core owns 1/8 of each bucket's rows),
token-parallel embeddings (128 tokens/core). No collectives — the host
assembles the full outputs from per-core shards.

Per-core device graph (identical across cores, SPMD):
  - all weights DMA'd into SBUF once (~13 MB bf16, resident)
  - emb   = gathered+masked rows (host gather) @ stacked proj weights
  - t_l^T = projs[l]^T @ hidden^T            (tail projections, transposed)
  - head logits = hidden @ [e0_shard | cls_w]^T  (cls_w folded in -> l_tail)
  - tail logits = t_l @ e_l_shard^T + l_tail[l-1] (bias fused into eviction)
  - logits evicted from PSUM with f32->bf16 cast, DMA'd out per 128-token row
"""

import numpy as np
import ml_dtypes

import concourse.bacc as bacc
import concourse.mybir as mybir
import concourse.tile as tile
from concourse.bass_utils import run_bass_kernel_spmd

BF16 = ml_dtypes.bfloat16
DT_BF16 = mybir.dt.bfloat16
DT_F32 = mybir.dt.float32

# ---- problem constants (hardcoded; kernel.py must be self-contained) ----
DIM = 1024
VOCABS = [20000, 20000, 30000, 30000]
VDS = [1024, 256, 64, 16]
VOFF = [0, 20000, 40000, 70000]
B, S = 2, 512
NTOK = B * S  # 1024
NCORES = 8
TPC = NTOK // NCORES  # 128 tokens/core for embeddings
VS = [v // NCORES for v in VOCABS]  # per-core vocab shard: 2500,2500,3750,3750
HEADC = VS[0] + 4  # head matmul cols: 2500 vocab + 3 cls + 1 pad
LCOLS = sum(VS)  # 12500 logit cols per core
KG = 11  # gather K-chunks: 1360 -> pad 1408 = 11*128
KPAD = KG * 128
GCOL0 = [0, 1024, 1280, 1344]  # col starts of each bucket in gathered G
PCOL0 = [None, 0, 256, 320]  # col starts of projs[l] in pall (width 336)
PALLC = 336

TRACE = False
LAST_EXEC_TIME_NS = None
LAST_TRACE_PATH = None

_CACHE = {}


def _install_trace_shim():
    """Register the NTFF profile hook that bass_utils expects under axon.

    The agent image's antenv package lacks ``axon_hooks``; provide an
    in-process stand-in and register the ctypes-driven hook from
    trn_agent_boot. Also neutralize the artifact upload (no bucket here).
    """
    import sys
    import types

    if "antenv.axon_hooks" not in sys.modules:
        mod = types.ModuleType("antenv.axon_hooks")
        _hook = [None]
        mod.set_axon_ntff_profile_hook = lambda h: _hook.__setitem__(0, h)
        mod.get_axon_ntff_profile_hook = lambda: _hook[0]
        sys.modules["antenv.axon_hooks"] = mod
        try:
            from trn_agent_boot.trn_boot import _ntff_profile_via_ctypes

            mod.set_axon_ntff_profile_hook(
                _ntff_profile_via_ctypes("/opt/axon/libaxon_pjrt.so")
            )
        except Exception:
            pass
    import concourse.bass_utils as _bu

    _bu.upload_artifacts = lambda tmpdir: tmpdir


def _chunks(total, size):
    return [(o, min(size, total - o)) for o in range(0, total, size)]


def _build(cls_b):
    nc = bacc.Bacc(None, target_bir_lowering=False, debug=False)

    d_hT = nc.declare_dram_parameter("hT", [128, 8, NTOK], DT_BF16, False)
    d_e0T = nc.declare_dram_parameter("e0T", [128, 8, HEADC], DT_BF16, False)
    d_e1T = nc.declare_dram_parameter("e1T", [128, 2, VS[1]], DT_BF16, False)
    d_e2T = nc.declare_dram_parameter("e2T", [64, VS[2]], DT_BF16, False)
    d_e3T = nc.declare_dram_parameter("e3T", [16, VS[3]], DT_BF16, False)
    d_pall = nc.declare_dram_parameter("pall", [128, 8, PALLC], DT_BF16, False)
    d_gT = nc.declare_dram_parameter("gT", [128, KG, TPC], DT_BF16, False)
    d_wemb = nc.declare_dram_parameter("wemb", [128, KG, DIM], DT_BF16, False)
    d_logits = nc.declare_dram_parameter("logits", [NTOK, LCOLS], DT_BF16, True)
    d_emb = nc.declare_dram_parameter("emb", [TPC, DIM], DT_F32, True)

    ADD = mybir.AluOpType.add

    with tile.TileContext(nc) as tc:
        with (
            tc.tile_pool(name="w", bufs=1) as wpool,
            tc.tile_pool(name="k", bufs=1) as kpool,
            tc.tile_pool(name="stage", bufs=2) as spool,
            tc.tile_pool(name="ps", bufs=8, space="PSUM") as pspool,
        ):
            hT = wpool.tile([128, 8, NTOK], DT_BF16)
            e0T = wpool.tile([128, 8, HEADC], DT_BF16)
            e1T = wpool.tile([128, 2, VS[1]], DT_BF16)
            e2T = wpool.tile([64, VS[2]], DT_BF16)
            e3T = wpool.tile([16, VS[3]], DT_BF16)
            pall = wpool.tile([128, 8, PALLC], DT_BF16)
            gT = wpool.tile([128, KG, TPC], DT_BF16)
            wemb = wpool.tile([128, KG, DIM], DT_BF16)

            t1 = kpool.tile([128, 2, NTOK], DT_BF16)
            t2 = kpool.tile([64, NTOK], DT_BF16)
            t3 = kpool.tile([16, NTOK], DT_BF16)
            ltail = kpool.tile([128, 8, 4], DT_F32)
            clsb = kpool.tile([128, 4], DT_F32)
            embsb = kpool.tile([TPC, DIM], DT_F32)

            # input DMAs on the SP HWDGE queue, ordered so early-phase deps
            # arrive first (outputs go on the ACT/Pool queues to not contend)
            nc.sync.dma_start(gT[:], d_gT[:])
            nc.sync.dma_start(wemb[:], d_wemb[:])
            nc.sync.dma_start(hT[:], d_hT[:])
            nc.sync.dma_start(pall[:], d_pall[:])
            nc.sync.dma_start(e0T[:], d_e0T[:])
            nc.sync.dma_start(e1T[:], d_e1T[:])
            nc.sync.dma_start(e2T[:], d_e2T[:])
            nc.sync.dma_start(e3T[:], d_e3T[:])

            for j in range(3):
                nc.vector.memset(clsb[:, j : j + 1], float(cls_b[j]))
            nc.vector.memset(clsb[:, 3:4], 0.0)

            # PE warmup: dense dummy matmuls so the HAM clock-gate reaches
            # 8/8 while the input DMAs stream in (PE would otherwise idle
            # cold for ~12us and pay the 1.2 GHz throttle on real work).
            wdum = kpool.tile([128, 512], DT_BF16)
            wsink = kpool.tile([128, 512], DT_F32)
            nc.vector.memset(wdum[:], 0.0)
            wps = pspool.tile([128, 512], DT_F32, tag="ps")
            for i in range(12):
                nc.tensor.matmul(wps[:], wdum[:, :128], wdum[:], start=True, stop=True)
            nc.vector.tensor_copy(wsink[:], wps[:])

            # ---- embeddings: emb = G @ W  (K = 1408, M = 128 tok, N = 1024)
            for n in range(2):
                ps = pspool.tile([128, 512], DT_F32, tag="ps")
                for k in range(KG):
                    nc.tensor.matmul(
                        ps[:],
                        gT[:, k, :],
                        wemb[:, k, n * 512 : (n + 1) * 512],
                        start=(k == 0),
                        stop=(k == KG - 1),
                    )
                nc.vector.tensor_copy(embsb[:, n * 512 : (n + 1) * 512], ps[:])
            nc.sync.dma_start(d_emb[:], embsb[:])

            # ---- t_l^T = projs[l]^T @ hidden^T  -> [vd_l, NTOK] bf16
            for mj in range(2):  # l=1, vd=256 -> two 128-row chunks
                for n in range(2):
                    ps = pspool.tile([128, 512], DT_F32, tag="ps")
                    for k in range(8):
                        nc.tensor.matmul(
                            ps[:],
                            pall[:, k, mj * 128 : (mj + 1) * 128],
                            hT[:, k, n * 512 : (n + 1) * 512],
                            start=(k == 0),
                            stop=(k == 7),
                        )
                    nc.vector.tensor_copy(t1[:, mj, n * 512 : (n + 1) * 512], ps[:])
            for tdst, lo, hi in ((t2, 256, 320), (t3, 320, 336)):
                mrows = hi - lo
                for n in range(2):
                    ps = pspool.tile([128, 512], DT_F32, tag="ps")
                    for k in range(8):
                        nc.tensor.matmul(
                            ps[:mrows, :],
                            pall[:, k, lo:hi],
                            hT[:, k, n * 512 : (n + 1) * 512],
                            start=(k == 0),
                            stop=(k == 7),
                        )
                    nc.vector.tensor_copy(tdst[:, n * 512 : (n + 1) * 512], ps[:mrows, :])

            # ---- main logits loop, one 128-token row-block at a time
            hchunks = [(2000, 504)] + _chunks(2000, 500)  # cls chunk first
            for m in range(8):
                ms = slice(m * 128, (m + 1) * 128)
                stg_h = spool.tile([128, VS[0]], DT_BF16, tag="stage_h")
                stg_t = spool.tile([128, LCOLS - VS[0]], DT_BF16, tag="stage_t")

                # head (+ fold-in of cls_w -> l_tail)
                for off, n_ in hchunks:
                    ps = pspool.tile([128, 512], DT_F32, tag="ps")
                    for k in range(8):
                        nc.tensor.matmul(
                            ps[:, :n_],
                            hT[:, k, ms],
                            e0T[:, k, off : off + n_],
                            start=(k == 0),
                            stop=(k == 7),
                        )
                    if n_ == 504:
                        nc.vector.tensor_copy(stg_h[:, 2000:2500], ps[:, :500])
                        nc.vector.tensor_tensor(ltail[:, m, :], ps[:, 500:504], clsb[:], op=ADD)
                    else:
                        nc.vector.tensor_copy(stg_h[:, off : off + n_], ps[:, :n_])
                nc.sync.dma_start(d_logits[ms, 0 : VS[0]], stg_h[:])

                # tail l=1 (K=256): bias add fused into eviction, engines alternate
                for i, (off, n_) in enumerate(_chunks(VS[1], 500)):
                    ps = pspool.tile([128, 512], DT_F32, tag="ps")
                    for k in range(2):
                        nc.tensor.matmul(
                            ps[:, :n_],
                            t1[:, k, ms],
                            e1T[:, k, off : off + n_],
                            start=(k == 0),
                            stop=(k == 1),
                        )
                    dst = stg_t[:, off : off + n_]
                    if i % 2 == 0:
                        nc.scalar.add(dst, ps[:, :n_], ltail[:, m, 0:1])
                    else:
                        nc.vector.tensor_scalar_add(dst, ps[:, :n_], ltail[:, m, 0:1])
                nc.sync.dma_start(d_logits[ms, VS[0] : VS[0] + VS[1]], stg_t[:, 0 : VS[1]])

                # tails l=2,3 (K=64/16): single matmul per chunk
                for tsrc, esrc, base, bcol in ((t2, e2T, 2500, 1), (t3, e3T, 6250, 2)):
                    for i, (off, n_) in enumerate(_chunks(VS[2], 512)):
                        ps = pspool.tile([128, 512], DT_F32, tag="ps")
                        nc.tensor.matmul(
                            ps[:, :n_], tsrc[:, ms], esrc[:, off : off + n_], start=True, stop=True
                        )
                        dst = stg_t[:, base + off : base + off + n_]
                        if i % 2 == 0:
                            nc.vector.tensor_scalar_add(dst, ps[:, :n_], ltail[:, m, bcol : bcol + 1])
                        else:
                            nc.scalar.add(dst, ps[:, :n_], ltail[:, m, bcol : bcol + 1])
                    nc.sync.dma_start(
                        d_logits[ms, VS[0] + base : VS[0] + base + VS[2]],
                        stg_t[:, base : base + VS[2]],
                    )

    nc.finalize()
    return nc


def _arrange(a2d, nk):
    """[nk*128, C] row-major -> [128, nk, C] (partition-major for SBUF)."""
    c = a2d.shape[1]
    return np.ascontiguousarray(a2d.reshape(nk, 128, c).transpose(1, 0, 2))


def kernel(input, hidden, embeds, projs, cls_w, cls_b):
    idx = np.asarray(input).astype(np.int64).reshape(-1)
    hidden = np.asarray(hidden, dtype=np.float32).reshape(NTOK, DIM)
    embeds = [np.asarray(e, dtype=np.float32) for e in embeds]
    projs = [np.asarray(p, dtype=np.float32) for p in projs]
    cls_w = np.asarray(cls_w, dtype=np.float32)
    cls_b = np.asarray(cls_b, dtype=np.float32)

    emb_bf = [e.astype(BF16) for e in embeds]
    proj_bf = [p.astype(BF16) for p in projs]
    cls_w_bf = cls_w.astype(BF16)

    # shared across cores
    hT_a = _arrange(np.ascontiguousarray(hidden.T).astype(BF16), 8)
    pall = np.zeros((DIM, PALLC), BF16)
    pall[:, 0:256] = proj_bf[1]
    pall[:, 256:320] = proj_bf[2]
    pall[:, 320:336] = proj_bf[3]
    pall_a = _arrange(pall, 8)
    W = np.zeros((KPAD, DIM), BF16)
    for l in range(4):
        W[GCOL0[l] : GCOL0[l] + VDS[l]] = proj_bf[l].T
    wemb_a = _arrange(W, KG)

    # host-side bucket-routed gather of raw embedding rows (masked, padded)
    bucket = np.searchsorted(np.asarray(VOFF[1:] + [100000]), idx, side="right")
    G = np.zeros((NTOK, KPAD), BF16)
    for l in range(4):
        sel = bucket == l
        rows = np.clip(idx - VOFF[l], 0, VOCABS[l] - 1)
        g = emb_bf[l][rows]
        g[~sel] = 0
        G[:, GCOL0[l] : GCOL0[l] + VDS[l]] = g

    key = cls_b.tobytes()
    if key not in _CACHE:
        _CACHE[key] = _build(cls_b)
    nc = _CACHE[key]

    in_maps = []
    for c in range(NCORES):
        E0ext = np.concatenate(
            [emb_bf[0][VS[0] * c : VS[0] * (c + 1)], cls_w_bf, np.zeros((1, DIM), BF16)], axis=0
        )  # [2504, DIM]
        in_maps.append(
            {
                "hT": hT_a,
                "e0T": _arrange(np.ascontiguousarray(E0ext.T), 8),
                "e1T": _arrange(
                    np.ascontiguousarray(emb_bf[1][VS[1] * c : VS[1] * (c + 1)].T), 2
                ),
                "e2T": np.ascontiguousarray(emb_bf[2][VS[2] * c : VS[2] * (c + 1)].T),
                "e3T": np.ascontiguousarray(emb_bf[3][VS[3] * c : VS[3] * (c + 1)].T),
                "pall": pall_a,
                "gT": _arrange(np.ascontiguousarray(G[TPC * c : TPC * (c + 1)].T), KG),
                "wemb": wemb_a,
            }
        )

    global LAST_EXEC_TIME_NS, LAST_TRACE_PATH
    if TRACE:
        _install_trace_shim()
    res = run_bass_kernel_spmd(nc, in_maps, core_ids=list(range(NCORES)), trace=TRACE)
    LAST_EXEC_TIME_NS = res.exec_time_ns
    if res.instructions_and_trace is not None:
        LAST_TRACE_PATH = res.instructions_and_trace[1]

    logits = np.empty((NTOK, sum(VOCABS)), np.float32)
    emb = np.empty((NTOK, DIM), np.float32)
    for c in range(NCORES):
        lg = np.asarray(res.results[c]["logits"]).astype(np.float32)
        logits[:, VS[0] * c : VS[0] * (c + 1)] = lg[:, 0 : VS[0]]
        logits[:, VOFF[1] + VS[1] * c : VOFF[1] + VS[1] * (c + 1)] = lg[:, VS[0] : VS[0] + VS[1]]
        logits[:, VOFF[2] + VS[2] * c : VOFF[2] + VS[2] * (c + 1)] = lg[:, 5000:8750]
        logits[:, VOFF[3] + VS[3] * c : VOFF[3] + VS[3] * (c + 1)] = lg[:, 8750:12500]
        emb[TPC * c : TPC * (c + 1)] = np.asarray(res.results[c]["emb"])
    return emb.reshape(B, S, DIM), logits.reshape(B, S, sum(VOCABS))
